# revision 2
# baseline (speedup 1.0000x reference)
"""GQA attention block (RoPE + causal softmax + out-projection) on 8 TRN2 cores.

Problem: q (2, 2048, 1024) 16 heads, k/v (2, 2048, 256) 4 kv heads (GQA rep 4),
causal attention, out @ w_out (1024, 1024).

Sharding: core c = (batch b = c//4, kv group = c%4). Each core computes its 4
q-heads x full T attention against its kv head, then the partial projection
X_heads @ w_out[head_rows, :]; the host sums the 4 partials per batch.

Structure (v2): RoPE applied host-side in f32. Heads processed in PAIRS as
virtual groups vg=(qcol-group g, head-pair hp): S psum tile (128, 2, 512)
holds both heads (each head's half = one psum bank), filled by two
64-contraction matmuls, consumed by ONE wide exp activation. Causal diag is a
post-exp binary-mask multiply (DVE) off the critical path: PV emission runs
one iteration behind S/exp so the PE queue never blocks on the current exp.
PV consumes P in fp8 (exp writes float8e4) via DoubleRow matmuls that reduce
over kb PAIRS at 0.5 cycles/row; [V|1] fp8 stationary, row 64 = softmax
denominator (fp8 quantization largely cancels in P/den). oT psum pairs
alternate A/B banks across vgs; the inactive pair hosts projection chunks of
the previous group, so psum-tag rotations only point backward. Projection is
bf16 (fp8 there costs too much accuracy), copied out via DVE and DMA'd as
bf16; host sums partials in f32.
"""

import sys

if "/opt/trn_rl_repo" not in sys.path:
    sys.path.insert(0, "/opt/trn_rl_repo")

import numpy as np

B, T, D, NH, NKV, HD = 2, 2048, 1024, 16, 4, 64
HC = NH // NKV          # q heads per core = 4
CD = HC * HD            # per-core channel dim = 256
NCORES = 8
QB = 128                # k block
NKB = T // QB
NG = T // 512           # q column groups of 512
GW = 512                # q group width

PV_FP8 = False

_cache: dict = {}


def _host_rope(x):
    """Apply RoPE to (B, T, n*64) in f32 on the host (matches reference)."""
    i = np.arange(32)
    ts = 10000.0 ** (i / 32.0)
    t = np.arange(T)
    ang = t[:, None] / ts[None, :]          # (T, 32)
    cos = np.cos(ang).astype(np.float32)
    sin = np.sin(ang).astype(np.float32)
    cos64 = np.concatenate([cos, cos], 1)   # (T, 64)
    sin64 = np.concatenate([sin, sin], 1)
    xh = x.reshape(*x.shape[:-1], -1, HD)   # (..., T, nh, 64)
    rot = np.concatenate([-xh[..., 32:], xh[..., :32]], axis=-1)
    out = xh * cos64[:, None, :] + rot * sin64[:, None, :]
    return out.reshape(x.shape)


def _build():
    import concourse.tile as tile
    from concourse import bacc, mybir

    f32 = mybir.dt.float32
    bf16 = mybir.dt.bfloat16
    fp8 = mybir.dt.float8e4
    DR = mybir.MatmulPerfMode.DoubleRow
    Exp = mybir.ActivationFunctionType.Exp
    pdt = fp8 if PV_FP8 else bf16

    nc = bacc.Bacc("TRN2", target_bir_lowering=False, debug=False,
                   num_devices=NCORES)

    d_qT = [nc.dram_tensor(f"qT{i}", [128, T], bf16, kind="ExternalInput")
            for i in range(2)]
    d_kT = nc.dram_tensor("kT", [128, T], bf16, kind="ExternalInput")
    # V stationary padded to 128 cols so FWL (fast weight load) triggers;
    # rows 65-127 of each PV output are junk and ignored
    d_vaug = nc.dram_tensor("vaug", [128, NKB, 128], pdt,
                            kind="ExternalInput")
    d_w = nc.dram_tensor("w", [CD, D], bf16, kind="ExternalInput")
    d_mask = nc.dram_tensor("maskb", [QB, 2, QB], pdt, kind="ExternalInput")
    d_out = nc.dram_tensor("outT", [D, T], bf16, kind="ExternalOutput")

    with tile.TileContext(nc) as tc:
        with (
            tc.tile_pool(name="data", bufs=1) as data,
            tc.tile_pool(name="pt", bufs=4) as ptp,
            tc.tile_pool(name="small", bufs=4) as small,
            tc.tile_pool(name="ps", bufs=2, space="PSUM") as ps,
            tc.tile_pool(name="psO", bufs=1, space="PSUM") as psO,
        ):
            qT = [data.tile([128, T], bf16, name=f"qT{i}", tag=f"qT{i}")
                  for i in range(2)]
            kT = data.tile([128, T], bf16, tag="kT")
            maskb = data.tile([QB, 2, QB], pdt, tag="maskb")
            vaug = data.tile([128, NKB, 128], pdt, tag="vaug")
            w = [data.tile([128, D], bf16, name=f"w{i}", tag=f"w{i}")
                 for i in range(2)]
            # SP-queue DMA issue costs ~1us each; put only the tensors the
            # first vg needs on SP, the rest on the (startup-idle) ACT queue
            nc.sync.dma_start(kT[:], d_kT[:])
            nc.sync.dma_start(qT[0][:], d_qT[0][:])
            nc.sync.dma_start(maskb[:], d_mask[:])
            nc.sync.dma_start(vaug[:], d_vaug[:])
            nc.scalar.dma_start(qT[1][:], d_qT[1][:])
            for i in range(2):
                nc.scalar.dma_start(w[i][:], d_w[i * 128:(i + 1) * 128, :])
            xT = [data.tile([128, T], bf16, name=f"xT{i}", tag=f"xT{i}")
                  for i in range(2)]
            ebias = data.tile([128, 1], f32, tag="ebias")
            nc.gpsimd.memset(ebias[:], -2.0 if PV_FP8 else 0.0)

            def emit_proj(qlo, n, pr, dma_engine=None, copy_engine=None):
                for cc in range(2):
                    nc.tensor.matmul(
                        pr[:], w[cc][:, n * 128:(n + 1) * 128],
                        xT[cc][:, qlo:qlo + GW],
                        start=(cc == 0), stop=(cc == 1))
                st = ptp.tile([128, GW], bf16, name="st", tag="st", bufs=6)
                if copy_engine is nc.scalar:
                    nc.scalar.copy(st[:], pr[:])
                else:
                    nc.vector.tensor_copy(st[:], pr[:])
                (dma_engine or nc.sync).dma_start(
                    d_out[n * 128:(n + 1) * 128, qlo:qlo + GW], st[:])

            # virtual group v = (g, hp): one head-pair's full kb loop.
            # oT psum pairs alternate A/B across vgs; the inactive pair's
            # banks host projection chunks of the previous logical group,
            # so all psum-tag rotations only ever point backward.
            pend: list[tuple[int, int]] = []

            def take_proj(v):
                other = "BA"[v % 2]
                tk = pend[:4]
                del pend[:4]
                prs = [psO.tile([128, GW], f32, name="pr",
                                tag=f"{other}{i % 2}")
                       for i in range(len(tk))]
                return list(zip(tk, prs))

            # The previous vg's epilogue (its last PV + normalization) is
            # deferred until after the NEXT vg's first S/exp emission, so the
            # ACT queue never idles across a vg boundary. PV emission runs
            # one iteration behind S/exp so the in-order PE queue never
            # blocks on the current exp; each pending PV carries its own
            # target oT tiles.
            epilogue = None
            pv_pend = None

            def flush_pv():
                nonlocal pv_pend
                if pv_pend is None:
                    return
                kb_, jmin_, PT_, oTs_, nkb_ = pv_pend
                pv_pend = None
                for hh in range(2):
                    nc.tensor.matmul(
                        oTs_[hh][:, jmin_:],
                        vaug[:, kb_, :],
                        PT_[:, hh, jmin_:],
                        start=(kb_ == 0), stop=(kb_ == nkb_ - 1),
                        skip_group_check=True)

            def make_epilogue(oTs, qlo, hp):
                def ep():
                    flush_pv()
                    # normalize straight out of psum; row 64 = denominator
                    for hh in range(2):
                        den_raw = small.tile([1, GW], f32, name="den_raw",
                                             tag="den_raw")
                        nc.vector.tensor_copy(den_raw[:],
                                              oTs[hh][HD:HD + 1, :])
                        den = small.tile([1, GW], f32, name="den", tag="den")
                        nc.vector.reciprocal_approx_fast(den[:], den_raw[:])
                        bcs = small.tile([64, GW], f32, name="bcs",
                                         tag="bcs")
                        nc.gpsimd.partition_broadcast(bcs[:], den[:])
                        nc.vector.tensor_mul(
                            xT[hp][hh * 64:(hh + 1) * 64, qlo:qlo + GW],
                            oTs[hh][:HD, :], bcs[:])
                return ep

            for v in range(2 * NG):
                g, hp = divmod(v, 2)
                qlo = g * GW
                nkb = (qlo + GW) // QB
                pair = "AB"[v % 2]
                chunks = take_proj(v)
                oTs = [psO.tile([128, GW], f32, name=f"oT{hh}",
                                tag=f"{pair}{hh}") for hh in range(2)]

                for kb in range(nkb):
                    jmin = max(0, kb * QB - qlo)
                    S = ps.tile([128, 2, GW], f32, name="S", tag="S")
                    for hh in range(2):
                        po = hh * 64
                        nc.tensor.matmul(
                            S[:, hh, jmin:],
                            kT[po:po + 64, kb * QB:(kb + 1) * QB],
                            qT[hp][po:po + 64, qlo + jmin:qlo + GW],
                            start=True, stop=True)
                    flush_pv()
                    PT = ptp.tile([128, 2, GW], pdt, name="PT", tag="PT")
                    nc.scalar.activation(PT[:, :, jmin:], S[:, :, jmin:],
                                         Exp, scale=0.125, bias=ebias[:])
                    if kb >= 4 * g:
                        nc.vector.tensor_mul(
                            PT[:, :, jmin:jmin + QB],
                            PT[:, :, jmin:jmin + QB], maskb[:])
                    if kb == 0 and epilogue is not None:
                        epilogue()
                        epilogue = None
                    elif chunks:
                        (pqlo, n), pr = chunks.pop(0)
                        emit_proj(pqlo, n, pr)
                    pv_pend = (kb, jmin, PT, oTs, nkb)
                while chunks:
                    (pqlo, n), pr = chunks.pop(0)
                    emit_proj(pqlo, n, pr)

                epilogue = make_epilogue(oTs, qlo, hp)
                if hp == 1:
                    pend += [(qlo, n) for n in range(D // 128)]

            epilogue()

            # tail: last group's projection — spread the 8 chunks over both
            # oT pairs AND the now-free S-pool slots for more parallelism
            tails = []
            for i, (pqlo, n) in enumerate(pend[:4]):
                tails.append(((pqlo, n),
                              psO.tile([128, GW], f32, name="pr",
                                       tag="AB"[i % 2] + str(i // 2))))
            for (pqlo, n) in pend[4:]:
                tails.append(((pqlo, n),
                              ps.tile([128, 2, GW], f32, name="prS",
                                      tag="S")[:, 0, :]))
            del pend[:]
            for i, ((pqlo, n), pr) in enumerate(tails):
                emit_proj(pqlo, n, pr,
                          dma_engine=nc.scalar if i % 2 else nc.sync,
                          copy_engine=nc.scalar if i % 2 else nc.vector)

    nc.finalize()
    return nc


def _get_nc():
    if "nc" not in _cache:
        _cache["nc"] = _build()
    return _cache["nc"]


def _in_maps(q, k, v, w_out):
    import ml_dtypes
    bf = ml_dtypes.bfloat16
    p8 = ml_dtypes.float8_e4m3 if PV_FP8 else bf

    qr = _host_rope(np.asarray(q, np.float32))
    kr = _host_rope(np.asarray(k, np.float32))
    kk = np.arange(QB)
    tri = (kk[:, None] <= kk[None, :]).astype(np.float32)  # (k_row, q_col)
    maskb = np.ascontiguousarray(
        np.broadcast_to(tri[:, None, :], (QB, 2, QB))).astype(p8)
    ones = np.ones((T, 1), np.float32)
    maps = []
    for c in range(NCORES):
        b, kv = divmod(c, NKV)
        kTc = np.ascontiguousarray(kr[b, :, kv * HD:(kv + 1) * HD].T)
        kT128 = np.concatenate([kTc, kTc], axis=0)  # replicate at parts 64-127
        va = np.concatenate([v[b, :, kv * HD:(kv + 1) * HD], ones,
                            np.zeros((T, 63), np.float32)], 1)
        maps.append({
            "qT0": np.ascontiguousarray(
                qr[b, :, kv * CD:kv * CD + 128].T).astype(bf),
            "qT1": np.ascontiguousarray(
                qr[b, :, kv * CD + 128:(kv + 1) * CD].T).astype(bf),
            "kT": kT128.astype(bf),
            "vaug": np.ascontiguousarray(
                va.reshape(NKB, QB, 128).transpose(1, 0, 2)).astype(p8),
            "w": np.ascontiguousarray(w_out[kv * CD:(kv + 1) * CD, :]).astype(bf),
            "maskb": maskb,
        })
    return maps


def _run(q, k, v, w_out, trace=False):
    from concourse.bass_utils import run_bass_kernel_spmd

    nc = _get_nc()
    res = run_bass_kernel_spmd(nc, _in_maps(q, k, v, w_out),
                               core_ids=list(range(NCORES)), trace=trace)
    out = np.zeros((B, T, D), np.float32)
    for c in range(NCORES):
        out[c // NKV] += res.results[c]["outT"].T.astype(np.float32)
    return out, res


def kernel(q, k, v, w_out):
    out, _ = _run(np.asarray(q), np.asarray(k), np.asarray(v),
                  np.asarray(w_out))
    return out


# revision 3
# speedup vs baseline: 1.0473x; 1.0473x over previous
"""GQA attention block (RoPE + causal softmax + out-projection) on 8 TRN2 cores.

Problem: q (2, 2048, 1024) 16 heads, k/v (2, 2048, 256) 4 kv heads (GQA rep 4),
causal attention, out @ w_out (1024, 1024).

Sharding: core c = (batch b = c//4, kv group = c%4). Each core computes its 4
q-heads x full T attention against its kv head, then the partial projection
X_heads @ w_out[head_rows, :]; the host sums the 4 partials per batch.

Structure (v2): RoPE applied host-side in f32. Heads processed in PAIRS as
virtual groups vg=(qcol-group g, head-pair hp): S psum tile (128, 2, 512)
holds both heads (each head's half = one psum bank), filled by two
64-contraction matmuls, consumed by ONE wide exp activation. Causal diag is a
post-exp binary-mask multiply (DVE) off the critical path: PV emission runs
one iteration behind S/exp so the PE queue never blocks on the current exp.
PV consumes P in fp8 (exp writes float8e4) via DoubleRow matmuls that reduce
over kb PAIRS at 0.5 cycles/row; [V|1] fp8 stationary, row 64 = softmax
denominator (fp8 quantization largely cancels in P/den). oT psum pairs
alternate A/B banks across vgs; the inactive pair hosts projection chunks of
the previous group, so psum-tag rotations only point backward. Projection is
bf16 (fp8 there costs too much accuracy), copied out via DVE and DMA'd as
bf16; host sums partials in f32.
"""

import sys

if "/opt/trn_rl_repo" not in sys.path:
    sys.path.insert(0, "/opt/trn_rl_repo")

import numpy as np

B, T, D, NH, NKV, HD = 2, 2048, 1024, 16, 4, 64
HC = NH // NKV          # q heads per core = 4
CD = HC * HD            # per-core channel dim = 256
NCORES = 8
QB = 128                # k block
NKB = T // QB
NG = T // 512           # q column groups of 512
GW = 512                # q group width

PV_FP8 = False

_cache: dict = {}


def _host_rope(x):
    """Apply RoPE to (B, T, n*64) in f32 on the host (matches reference)."""
    i = np.arange(32)
    ts = 10000.0 ** (i / 32.0)
    t = np.arange(T)
    ang = t[:, None] / ts[None, :]          # (T, 32)
    cos = np.cos(ang).astype(np.float32)
    sin = np.sin(ang).astype(np.float32)
    cos64 = np.concatenate([cos, cos], 1)   # (T, 64)
    sin64 = np.concatenate([sin, sin], 1)
    xh = x.reshape(*x.shape[:-1], -1, HD)   # (..., T, nh, 64)
    rot = np.concatenate([-xh[..., 32:], xh[..., :32]], axis=-1)
    out = xh * cos64[:, None, :] + rot * sin64[:, None, :]
    return out.reshape(x.shape)


def _build():
    import concourse.tile as tile
    from concourse import bacc, mybir

    f32 = mybir.dt.float32
    bf16 = mybir.dt.bfloat16
    fp8 = mybir.dt.float8e4
    DR = mybir.MatmulPerfMode.DoubleRow
    Exp = mybir.ActivationFunctionType.Exp
    pdt = fp8 if PV_FP8 else bf16

    nc = bacc.Bacc("TRN2", target_bir_lowering=False, debug=False,
                   num_devices=NCORES)

    d_qT = [nc.dram_tensor(f"qT{i}", [128, T], bf16, kind="ExternalInput")
            for i in range(2)]
    d_kT = nc.dram_tensor("kT", [128, T], bf16, kind="ExternalInput")
    # V stationary padded to 128 cols so FWL (fast weight load) triggers;
    # rows 65-127 of each PV output are junk and ignored
    d_vaug = nc.dram_tensor("vaug", [128, NKB, 128], pdt,
                            kind="ExternalInput")
    d_w = nc.dram_tensor("w", [CD, D], bf16, kind="ExternalInput")
    d_mask = nc.dram_tensor("maskb", [QB, 2, QB], pdt, kind="ExternalInput")
    d_out = nc.dram_tensor("outT", [D, T], bf16, kind="ExternalOutput")

    with tile.TileContext(nc) as tc:
        with (
            tc.tile_pool(name="data", bufs=1) as data,
            tc.tile_pool(name="pt", bufs=5) as ptp,
            tc.tile_pool(name="small", bufs=4) as small,
            tc.tile_pool(name="ps", bufs=2, space="PSUM") as ps,
            tc.tile_pool(name="psO", bufs=1, space="PSUM") as psO,
        ):
            qT = [data.tile([128, T], bf16, name=f"qT{i}", tag=f"qT{i}")
                  for i in range(2)]
            kT = data.tile([128, T], bf16, tag="kT")
            maskb = data.tile([QB, 2, QB], pdt, tag="maskb")
            vaug = data.tile([128, NKB, 128], pdt, tag="vaug")
            w = [data.tile([128, D], bf16, name=f"w{i}", tag=f"w{i}")
                 for i in range(2)]
            # SP-queue DMA issue costs ~1us each; put only the tensors the
            # first vg needs on SP, the rest on the (startup-idle) ACT queue
            nc.sync.dma_start(kT[:], d_kT[:])
            nc.sync.dma_start(qT[0][:], d_qT[0][:])
            nc.sync.dma_start(maskb[:], d_mask[:])
            nc.sync.dma_start(vaug[:], d_vaug[:])
            nc.scalar.dma_start(qT[1][:], d_qT[1][:])
            for i in range(2):
                nc.scalar.dma_start(w[i][:], d_w[i * 128:(i + 1) * 128, :])
            xT = [data.tile([128, T], bf16, name=f"xT{i}", tag=f"xT{i}")
                  for i in range(2)]
            ebias = data.tile([128, 1], f32, tag="ebias")
            nc.gpsimd.memset(ebias[:], -2.0 if PV_FP8 else 0.0)
            wexp = data.tile([128, 1], f32, tag="wexp")
            nc.scalar.activation(wexp[:], ebias[:], Exp, scale=1.0,
                                 bias=ebias[:])

            def emit_proj(qlo, n, pr, dma_engine=None, copy_engine=None):
                for cc in range(2):
                    nc.tensor.matmul(
                        pr[:], w[cc][:, n * 128:(n + 1) * 128],
                        xT[cc][:, qlo:qlo + GW],
                        start=(cc == 0), stop=(cc == 1))
                st = ptp.tile([128, GW], bf16, name="st", tag="st", bufs=6)
                if copy_engine is nc.scalar:
                    nc.scalar.copy(st[:], pr[:])
                else:
                    nc.vector.tensor_copy(st[:], pr[:])
                (dma_engine or nc.sync).dma_start(
                    d_out[n * 128:(n + 1) * 128, qlo:qlo + GW], st[:])

            # virtual group v = (g, hp): one head-pair's full kb loop.
            # oT psum pairs alternate A/B across vgs; the inactive pair's
            # banks host projection chunks of the previous logical group,
            # so all psum-tag rotations only ever point backward.
            pend: list[tuple[int, int]] = []

            def take_proj(v):
                other = "BA"[v % 2]
                tk = pend[:4]
                del pend[:4]
                prs = [psO.tile([128, GW], f32, name="pr",
                                tag=f"{other}{i % 2}")
                       for i in range(len(tk))]
                return list(zip(tk, prs))

            # The previous vg's epilogue (its last PVs + normalization) is
            # deferred until after the NEXT vg's first S/exp emission, so the
            # ACT queue never idles across a vg boundary. PV emission runs
            # TWO iterations behind S/exp: by the time a PV enters the
            # in-order PE queue, its exp finished two iterations ago, so the
            # PE never stalls on it; each pending PV carries its own target
            # oT tiles.
            epilogue = None
            pv_q = []

            def flush_one_pv():
                kb_, jmin_, PT_, oTs_, nkb_ = pv_q.pop(0)
                for hh in range(2):
                    nc.tensor.matmul(
                        oTs_[hh][:, jmin_:],
                        vaug[:, kb_, :],
                        PT_[:, hh, jmin_:],
                        start=(kb_ == 0), stop=(kb_ == nkb_ - 1),
                        skip_group_check=True)

            def flush_all_pv():
                while pv_q:
                    flush_one_pv()

            def make_epilogue(oTs, qlo, hp):
                def ep():
                    flush_all_pv()
                    # normalize straight out of psum; row 64 = denominator
                    for hh in range(2):
                        den_raw = small.tile([1, GW], f32, name="den_raw",
                                             tag="den_raw")
                        nc.vector.tensor_copy(den_raw[:],
                                              oTs[hh][HD:HD + 1, :])
                        den = small.tile([1, GW], f32, name="den", tag="den")
                        nc.vector.reciprocal_approx_fast(den[:], den_raw[:])
                        bcs = small.tile([64, GW], f32, name="bcs",
                                         tag="bcs")
                        nc.gpsimd.partition_broadcast(bcs[:], den[:])
                        nc.vector.tensor_mul(
                            xT[hp][hh * 64:(hh + 1) * 64, qlo:qlo + GW],
                            oTs[hh][:HD, :], bcs[:])
                return ep

            for v in range(2 * NG):
                g, hp = divmod(v, 2)
                qlo = g * GW
                nkb = (qlo + GW) // QB
                pair = "AB"[v % 2]
                chunks = take_proj(v)
                oTs = [psO.tile([128, GW], f32, name=f"oT{hh}",
                                tag=f"{pair}{hh}") for hh in range(2)]

                for kb in range(nkb):
                    jmin = max(0, kb * QB - qlo)
                    S = ps.tile([128, 2, GW], f32, name="S", tag="S")
                    for hh in range(2):
                        po = hh * 64
                        nc.tensor.matmul(
                            S[:, hh, jmin:],
                            kT[po:po + 64, kb * QB:(kb + 1) * QB],
                            qT[hp][po:po + 64, qlo + jmin:qlo + GW],
                            start=True, stop=True)
                    if len(pv_q) >= 2:
                        flush_one_pv()
                    PT = ptp.tile([128, 2, GW], pdt, name="PT", tag="PT")
                    nc.scalar.activation(PT[:, :, jmin:], S[:, :, jmin:],
                                         Exp, scale=0.125, bias=ebias[:])
                    if kb >= 4 * g:
                        nc.vector.tensor_mul(
                            PT[:, :, jmin:jmin + QB],
                            PT[:, :, jmin:jmin + QB], maskb[:])
                    if kb == 0 and epilogue is not None:
                        epilogue()
                        epilogue = None
                    elif chunks:
                        (pqlo, n), pr = chunks.pop(0)
                        emit_proj(pqlo, n, pr)
                    pv_q.append((kb, jmin, PT, oTs, nkb))
                while chunks:
                    (pqlo, n), pr = chunks.pop(0)
                    emit_proj(pqlo, n, pr)

                epilogue = make_epilogue(oTs, qlo, hp)
                if hp == 1:
                    pend += [(qlo, n) for n in range(D // 128)]

            epilogue()

            # tail: last group's projection — spread the 8 chunks over both
            # oT pairs AND the now-free S-pool slots for more parallelism
            tails = []
            for i, (pqlo, n) in enumerate(pend[:4]):
                tails.append(((pqlo, n),
                              psO.tile([128, GW], f32, name="pr",
                                       tag="AB"[i % 2] + str(i // 2))))
            for (pqlo, n) in pend[4:]:
                tails.append(((pqlo, n),
                              ps.tile([128, 2, GW], f32, name="prS",
                                      tag="S")[:, 0, :]))
            del pend[:]
            for i, ((pqlo, n), pr) in enumerate(tails):
                emit_proj(pqlo, n, pr,
                          dma_engine=nc.scalar if i % 2 else nc.sync,
                          copy_engine=nc.scalar if i % 2 else nc.vector)

    nc.finalize()
    return nc


def _get_nc():
    if "nc" not in _cache:
        _cache["nc"] = _build()
    return _cache["nc"]


def _in_maps(q, k, v, w_out):
    import ml_dtypes
    bf = ml_dtypes.bfloat16
    p8 = ml_dtypes.float8_e4m3 if PV_FP8 else bf

    qr = _host_rope(np.asarray(q, np.float32))
    kr = _host_rope(np.asarray(k, np.float32))
    kk = np.arange(QB)
    tri = (kk[:, None] <= kk[None, :]).astype(np.float32)  # (k_row, q_col)
    maskb = np.ascontiguousarray(
        np.broadcast_to(tri[:, None, :], (QB, 2, QB))).astype(p8)
    ones = np.ones((T, 1), np.float32)
    maps = []
    for c in range(NCORES):
        b, kv = divmod(c, NKV)
        kTc = np.ascontiguousarray(kr[b, :, kv * HD:(kv + 1) * HD].T)
        kT128 = np.concatenate([kTc, kTc], axis=0)  # replicate at parts 64-127
        va = np.concatenate([v[b, :, kv * HD:(kv + 1) * HD], ones,
                            np.zeros((T, 63), np.float32)], 1)
        maps.append({
            "qT0": np.ascontiguousarray(
                qr[b, :, kv * CD:kv * CD + 128].T).astype(bf),
            "qT1": np.ascontiguousarray(
                qr[b, :, kv * CD + 128:(kv + 1) * CD].T).astype(bf),
            "kT": kT128.astype(bf),
            "vaug": np.ascontiguousarray(
                va.reshape(NKB, QB, 128).transpose(1, 0, 2)).astype(p8),
            "w": np.ascontiguousarray(w_out[kv * CD:(kv + 1) * CD, :]).astype(bf),
            "maskb": maskb,
        })
    return maps


def _run(q, k, v, w_out, trace=False):
    from concourse.bass_utils import run_bass_kernel_spmd

    nc = _get_nc()
    res = run_bass_kernel_spmd(nc, _in_maps(q, k, v, w_out),
                               core_ids=list(range(NCORES)), trace=trace)
    out = np.zeros((B, T, D), np.float32)
    for c in range(NCORES):
        out[c // NKV] += res.results[c]["outT"].T.astype(np.float32)
    return out, res


def kernel(q, k, v, w_out):
    out, _ = _run(np.asarray(q), np.asarray(k), np.asarray(v),
                  np.asarray(w_out))
    return out


# revision 4
# speedup vs baseline: 1.0533x; 1.0057x over previous
"""GQA attention block (RoPE + causal softmax + out-projection) on 8 TRN2 cores.

Problem: q (2, 2048, 1024) 16 heads, k/v (2, 2048, 256) 4 kv heads (GQA rep 4),
causal attention, out @ w_out (1024, 1024).

Sharding: core c = (batch b = c//4, kv group = c%4). Each core computes its 4
q-heads x full T attention against its kv head, then the partial projection
X_heads @ w_out[head_rows, :]; the host sums the 4 partials per batch.

Structure (v2): RoPE applied host-side in f32. Heads processed in PAIRS as
virtual groups vg=(qcol-group g, head-pair hp): S psum tile (128, 2, 512)
holds both heads (each head's half = one psum bank), filled by two
64-contraction matmuls, consumed by ONE wide exp activation. Causal diag is a
post-exp binary-mask multiply (DVE) off the critical path: PV emission runs
one iteration behind S/exp so the PE queue never blocks on the current exp.
PV consumes P in fp8 (exp writes float8e4) via DoubleRow matmuls that reduce
over kb PAIRS at 0.5 cycles/row; [V|1] fp8 stationary, row 64 = softmax
denominator (fp8 quantization largely cancels in P/den). oT psum pairs
alternate A/B banks across vgs; the inactive pair hosts projection chunks of
the previous group, so psum-tag rotations only point backward. Projection is
bf16 (fp8 there costs too much accuracy), copied out via DVE and DMA'd as
bf16; host sums partials in f32.
"""

import sys

if "/opt/trn_rl_repo" not in sys.path:
    sys.path.insert(0, "/opt/trn_rl_repo")

import numpy as np

B, T, D, NH, NKV, HD = 2, 2048, 1024, 16, 4, 64
HC = NH // NKV          # q heads per core = 4
CD = HC * HD            # per-core channel dim = 256
NCORES = 8
QB = 128                # k block
NKB = T // QB
NG = T // 512           # q column groups of 512
GW = 512                # q group width

PV_FP8 = False

_cache: dict = {}


def _host_rope(x):
    """Apply RoPE to (B, T, n*64) in f32 on the host (matches reference)."""
    i = np.arange(32)
    ts = 10000.0 ** (i / 32.0)
    t = np.arange(T)
    ang = t[:, None] / ts[None, :]          # (T, 32)
    cos = np.cos(ang).astype(np.float32)
    sin = np.sin(ang).astype(np.float32)
    cos64 = np.concatenate([cos, cos], 1)   # (T, 64)
    sin64 = np.concatenate([sin, sin], 1)
    xh = x.reshape(*x.shape[:-1], -1, HD)   # (..., T, nh, 64)
    rot = np.concatenate([-xh[..., 32:], xh[..., :32]], axis=-1)
    out = xh * cos64[:, None, :] + rot * sin64[:, None, :]
    return out.reshape(x.shape)


def _build():
    import concourse.tile as tile
    from concourse import bacc, mybir

    f32 = mybir.dt.float32
    bf16 = mybir.dt.bfloat16
    fp8 = mybir.dt.float8e4
    DR = mybir.MatmulPerfMode.DoubleRow
    Exp = mybir.ActivationFunctionType.Exp
    pdt = fp8 if PV_FP8 else bf16

    nc = bacc.Bacc("TRN2", target_bir_lowering=False, debug=False,
                   num_devices=NCORES)

    d_qT = [nc.dram_tensor(f"qT{i}", [128, T], bf16, kind="ExternalInput")
            for i in range(2)]
    d_kT = nc.dram_tensor("kT", [128, T], bf16, kind="ExternalInput")
    # V stationary padded to 128 cols so FWL (fast weight load) triggers;
    # rows 65-127 of each PV output are junk and ignored
    d_vaug = nc.dram_tensor("vaug", [128, NKB, 128], pdt,
                            kind="ExternalInput")
    d_w = nc.dram_tensor("w", [CD, D], bf16, kind="ExternalInput")
    d_mask = nc.dram_tensor("maskb", [QB, 2, QB], pdt, kind="ExternalInput")
    d_out = nc.dram_tensor("outT", [D, T], bf16, kind="ExternalOutput")

    with tile.TileContext(nc) as tc:
        with (
            tc.tile_pool(name="data", bufs=1) as data,
            tc.tile_pool(name="pt", bufs=5) as ptp,
            tc.tile_pool(name="small", bufs=4) as small,
            tc.tile_pool(name="ps", bufs=2, space="PSUM") as ps,
            tc.tile_pool(name="psO", bufs=1, space="PSUM") as psO,
        ):
            qT = [data.tile([128, T], bf16, name=f"qT{i}", tag=f"qT{i}")
                  for i in range(2)]
            kT = data.tile([128, T], bf16, tag="kT")
            maskb = data.tile([QB, 2, QB], pdt, tag="maskb")
            vaug = data.tile([128, NKB, 128], pdt, tag="vaug")
            w = [data.tile([128, D], bf16, name=f"w{i}", tag=f"w{i}")
                 for i in range(2)]
            # SP-queue DMA issue costs ~1us each; put only the tensors the
            # first vg needs on SP, the rest on the (startup-idle) ACT queue
            nc.sync.dma_start(kT[:], d_kT[:])
            nc.sync.dma_start(qT[0][:], d_qT[0][:])
            nc.sync.dma_start(maskb[:], d_mask[:])
            nc.sync.dma_start(vaug[:], d_vaug[:])
            nc.scalar.dma_start(qT[1][:], d_qT[1][:])
            for i in range(2):
                nc.scalar.dma_start(w[i][:], d_w[i * 128:(i + 1) * 128, :])
            xT = [data.tile([128, T], bf16, name=f"xT{i}", tag=f"xT{i}")
                  for i in range(2)]
            ebias = data.tile([128, 1], f32, tag="ebias")
            nc.gpsimd.memset(ebias[:], -2.0 if PV_FP8 else 0.0)
            wexp = data.tile([128, 1], f32, tag="wexp")
            nc.scalar.activation(wexp[:], ebias[:], Exp, scale=1.0,
                                 bias=ebias[:])

            def emit_proj(qlo, n, pr, dma_engine=None, copy_engine=None):
                for cc in range(2):
                    nc.tensor.matmul(
                        pr[:], w[cc][:, n * 128:(n + 1) * 128],
                        xT[cc][:, qlo:qlo + GW],
                        start=(cc == 0), stop=(cc == 1))
                st = ptp.tile([128, GW], bf16, name="st", tag="st", bufs=6)
                if copy_engine is nc.scalar:
                    nc.scalar.copy(st[:], pr[:])
                else:
                    nc.vector.tensor_copy(st[:], pr[:])
                (dma_engine or nc.sync).dma_start(
                    d_out[n * 128:(n + 1) * 128, qlo:qlo + GW], st[:])

            # virtual group v = (g, hp): one head-pair's full kb loop.
            # oT psum pairs alternate A/B across vgs; the inactive pair's
            # banks host projection chunks of the previous logical group,
            # so all psum-tag rotations only ever point backward.
            pend: list[tuple[int, int]] = []

            def take_proj(v):
                other = "BA"[v % 2]
                tk = pend[:4]
                del pend[:4]
                prs = [psO.tile([128, GW], f32, name="pr",
                                tag=f"{other}{i % 2}")
                       for i in range(len(tk))]
                return list(zip(tk, prs))

            # The previous vg's epilogue (its last PVs + normalization) is
            # deferred until after the NEXT vg's first S/exp emission, so the
            # ACT queue never idles across a vg boundary. PV emission runs
            # TWO iterations behind S/exp: by the time a PV enters the
            # in-order PE queue, its exp finished two iterations ago, so the
            # PE never stalls on it; each pending PV carries its own target
            # oT tiles.
            epilogue = None
            pv_q = []

            def flush_one_pv():
                kb_, jmin_, PT_, oTs_, nkb_ = pv_q.pop(0)
                for hh in range(2):
                    nc.tensor.matmul(
                        oTs_[hh][:, jmin_:],
                        vaug[:, kb_, :],
                        PT_[:, hh, jmin_:],
                        start=(kb_ == 0), stop=(kb_ == nkb_ - 1),
                        skip_group_check=True)

            def flush_all_pv():
                while pv_q:
                    flush_one_pv()

            def make_epilogue(oTs, qlo, hp):
                def ep():
                    flush_all_pv()
                    # normalize straight out of psum; row 64 = denominator
                    for hh in range(2):
                        den_raw = small.tile([1, GW], f32, name="den_raw",
                                             tag="den_raw")
                        nc.vector.tensor_copy(den_raw[:],
                                              oTs[hh][HD:HD + 1, :])
                        den = small.tile([1, GW], f32, name="den", tag="den")
                        nc.vector.reciprocal_approx_fast(den[:], den_raw[:])
                        bcs = small.tile([64, GW], f32, name="bcs",
                                         tag="bcs")
                        nc.gpsimd.partition_broadcast(bcs[:], den[:])
                        nc.vector.tensor_mul(
                            xT[hp][hh * 64:(hh + 1) * 64, qlo:qlo + GW],
                            oTs[hh][:HD, :], bcs[:])
                return ep

            for v in range(2 * NG):
                g, hp = divmod(v, 2)
                qlo = g * GW
                nkb = (qlo + GW) // QB
                pair = "AB"[v % 2]
                chunks = take_proj(v)
                oTs = [psO.tile([128, GW], f32, name=f"oT{hh}",
                                tag=f"{pair}{hh}") for hh in range(2)]

                for kb in range(nkb):
                    jmin = max(0, kb * QB - qlo)
                    S = ps.tile([128, 2, GW], f32, name="S", tag="S")
                    for hh in range(2):
                        po = hh * 64
                        nc.tensor.matmul(
                            S[:, hh, jmin:],
                            kT[po:po + 64, kb * QB:(kb + 1) * QB],
                            qT[hp][po:po + 64, qlo + jmin:qlo + GW],
                            start=True, stop=True)
                    if len(pv_q) >= 2:
                        flush_one_pv()
                    PT = ptp.tile([128, 2, GW], pdt, name="PT", tag="PT")
                    nc.scalar.activation(PT[:, :, jmin:], S[:, :, jmin:],
                                         Exp, scale=0.125, bias=ebias[:])
                    if kb >= 4 * g:
                        nc.vector.tensor_mul(
                            PT[:, :, jmin:jmin + QB],
                            PT[:, :, jmin:jmin + QB], maskb[:])
                    if kb == 0 and epilogue is not None:
                        epilogue()
                        epilogue = None
                    elif chunks:
                        (pqlo, n), pr = chunks.pop(0)
                        emit_proj(pqlo, n, pr)
                    pv_q.append((kb, jmin, PT, oTs, nkb))
                    # last vg: drain PVs eagerly so the tail chain
                    # (norm -> projection) starts as early as possible
                    if v == 2 * NG - 1 and kb >= nkb - 2:
                        flush_all_pv()
                while chunks:
                    (pqlo, n), pr = chunks.pop(0)
                    emit_proj(pqlo, n, pr)

                epilogue = make_epilogue(oTs, qlo, hp)
                if hp == 1:
                    pend += [(qlo, n) for n in range(D // 128)]

            epilogue()

            # tail: last group's projection — spread the 8 chunks over both
            # oT pairs AND the now-free S-pool slots for more parallelism
            tails = []
            for i, (pqlo, n) in enumerate(pend[:4]):
                tails.append(((pqlo, n),
                              psO.tile([128, GW], f32, name="pr",
                                       tag="AB"[i % 2] + str(i // 2))))
            for (pqlo, n) in pend[4:]:
                tails.append(((pqlo, n),
                              ps.tile([128, 2, GW], f32, name="prS",
                                      tag="S")[:, 0, :]))
            del pend[:]
            for i, ((pqlo, n), pr) in enumerate(tails):
                emit_proj(pqlo, n, pr,
                          dma_engine=nc.scalar if i % 2 else nc.sync,
                          copy_engine=nc.scalar if i % 2 else nc.vector)

    nc.finalize()
    return nc


def _get_nc():
    if "nc" not in _cache:
        _cache["nc"] = _build()
    return _cache["nc"]


def _in_maps(q, k, v, w_out):
    import ml_dtypes
    bf = ml_dtypes.bfloat16
    p8 = ml_dtypes.float8_e4m3 if PV_FP8 else bf

    qr = _host_rope(np.asarray(q, np.float32))
    kr = _host_rope(np.asarray(k, np.float32))
    kk = np.arange(QB)
    tri = (kk[:, None] <= kk[None, :]).astype(np.float32)  # (k_row, q_col)
    maskb = np.ascontiguousarray(
        np.broadcast_to(tri[:, None, :], (QB, 2, QB))).astype(p8)
    ones = np.ones((T, 1), np.float32)
    maps = []
    for c in range(NCORES):
        b, kv = divmod(c, NKV)
        kTc = np.ascontiguousarray(kr[b, :, kv * HD:(kv + 1) * HD].T)
        kT128 = np.concatenate([kTc, kTc], axis=0)  # replicate at parts 64-127
        va = np.concatenate([v[b, :, kv * HD:(kv + 1) * HD], ones,
                            np.zeros((T, 63), np.float32)], 1)
        maps.append({
            "qT0": np.ascontiguousarray(
                qr[b, :, kv * CD:kv * CD + 128].T).astype(bf),
            "qT1": np.ascontiguousarray(
                qr[b, :, kv * CD + 128:(kv + 1) * CD].T).astype(bf),
            "kT": kT128.astype(bf),
            "vaug": np.ascontiguousarray(
                va.reshape(NKB, QB, 128).transpose(1, 0, 2)).astype(p8),
            "w": np.ascontiguousarray(w_out[kv * CD:(kv + 1) * CD, :]).astype(bf),
            "maskb": maskb,
        })
    return maps


def _run(q, k, v, w_out, trace=False):
    from concourse.bass_utils import run_bass_kernel_spmd

    nc = _get_nc()
    res = run_bass_kernel_spmd(nc, _in_maps(q, k, v, w_out),
                               core_ids=list(range(NCORES)), trace=trace)
    out = np.zeros((B, T, D), np.float32)
    for c in range(NCORES):
        out[c // NKV] += res.results[c]["outT"].T.astype(np.float32)
    return out, res


def kernel(q, k, v, w_out):
    out, _ = _run(np.asarray(q), np.asarray(k), np.asarray(v),
                  np.asarray(w_out))
    return out
